# revision 1
# baseline (speedup 1.0000x reference)
"""Sliding-window GQA attention (softcap + clipped softmax) on 8 TRN2 NeuronCores.

v2: fp32-matmul-penalty-aware restructure.
  - scores matmul in float32r (1 cyc/row at N>=256 vs 4 for fp32)
  - normalize+shift E on DVE in [q,k] layout (per-partition scalar, 1 pass)
  - transpose normalized fp16 tile via PE transpose-mode (1 cyc/row)
  - clamp fused into the PSUM->SBUF copy (1 DVE pass)
  - A@V in fp16

Sharding: core c -> batch c//4, GQA group c%4 (4 q-heads sharing 1 kv head).
"""

import sys

sys.path.insert(0, "/opt/trn_rl_repo")

import numpy as np

B = 2
S = 2048
HQ = 16
HKV = 4
D = 128
NB = S // 128
WB = 8
CAP = 30.0
SCALE = float(1.0 / np.float32(np.sqrt(np.float32(D))))
MASK_VAL = -1.0e4
SCORES_F32R = True

_CACHED = {}


def _chunks(wc):
    # PSUM-bank-aligned <=512 chunks (each matmul output within one bank)
    out = []
    while wc > 0:
        c = min(512, wc)
        out.append(c)
        wc -= c
    return out


def _build_bass():
    import concourse.mybir as mybir
    import concourse.tile as tile
    from concourse import bacc
    from contextlib import ExitStack

    f32 = mybir.dt.float32
    f32r = mybir.dt.float32r
    f16 = mybir.dt.float16
    AF = mybir.ActivationFunctionType
    OP = mybir.AluOpType

    nc = bacc.Bacc("TRN2", target_bir_lowering=False, debug=False, num_devices=8)

    qk_dt = f32r if SCORES_F32R else f32
    qT = nc.dram_tensor("qT", [4, 128, S], qk_dt, kind="ExternalInput").ap()
    kT = nc.dram_tensor("kT", [128, S], qk_dt, kind="ExternalInput").ap()
    vh = nc.dram_tensor("vh", [S, 128], f16, kind="ExternalInput").ap()
    msk = nc.dram_tensor("msk", [2, 128, 128], f32, kind="ExternalInput").ap()
    idn = nc.dram_tensor("idn", [128, 128], f16, kind="ExternalInput").ap()
    out = nc.dram_tensor("out", [S, 4, 128], f32, kind="ExternalOutput").ap()

    with tile.TileContext(nc) as tc:
        with ExitStack() as ctx:
            singles = ctx.enter_context(tc.tile_pool(name="singles", bufs=1))
            qpool = ctx.enter_context(tc.tile_pool(name="qpool", bufs=2))
            tpool = ctx.enter_context(tc.tile_pool(name="tpool", bufs=3))
            epool = ctx.enter_context(tc.tile_pool(name="epool", bufs=3))
            ypool = ctx.enter_context(tc.tile_pool(name="ypool", bufs=3))
            apool = ctx.enter_context(tc.tile_pool(name="apool", bufs=3))
            spool = ctx.enter_context(tc.tile_pool(name="spool", bufs=6))
            opool = ctx.enter_context(tc.tile_pool(name="opool", bufs=3))
            psco = ctx.enter_context(tc.tile_pool(name="psco", bufs=2, space="PSUM"))
            ptp = ctx.enter_context(tc.tile_pool(name="ptp", bufs=1, space="PSUM"))
            pout = ctx.enter_context(tc.tile_pool(name="pout", bufs=1, space="PSUM"))

            kT_sb = singles.tile([128, S], qk_dt)
            nc.sync.dma_start(kT_sb, kT)
            v_sb = singles.tile([128, NB, 128], f16)
            for j in range(NB):
                nc.sync.dma_start(v_sb[:, j, :], vh[j * 128:(j + 1) * 128, :])
            m_sb = singles.tile([128, 2, 128], f32)
            nc.sync.dma_start(m_sb, msk.rearrange("t p c -> p t c"))
            i_sb = singles.tile([128, 128], f16)
            nc.sync.dma_start(i_sb, idn)
            bcap = singles.tile([128, 1], f32)
            nc.gpsimd.memset(bcap, -CAP)

            for h in range(4):
                qT_sb = qpool.tile([128, S], qk_dt, tag="qt")
                nc.sync.dma_start(qT_sb, qT[h])
                for i in range(NB):
                    j0 = max(0, i - WB)
                    nW = i - j0 + 1
                    wc = nW * 128

                    ps_full = psco.tile([128, 1152], f32, tag="s")
                    ps = ps_full[:, :wc]
                    c0 = 0
                    for cw in _chunks(wc):
                        nc.tensor.matmul(
                            ps[:, c0:c0 + cw],
                            lhsT=qT_sb[:, i * 128:(i + 1) * 128],
                            rhs=kT_sb[:, j0 * 128 + c0: j0 * 128 + c0 + cw],
                            start=True,
                            stop=True,
                        )
                        c0 += cw

                    t_sb_full = tpool.tile([128, 1152], f32, tag="t")
                    t_sb = t_sb_full[:, :wc]
                    nc.scalar.activation(t_sb, ps, AF.Tanh, scale=SCALE)

                    nc.gpsimd.tensor_tensor(
                        t_sb[:, (nW - 1) * 128: nW * 128],
                        t_sb[:, (nW - 1) * 128: nW * 128],
                        m_sb[:, 0, :],
                        op=OP.add,
                    )
                    if i >= WB:
                        nc.gpsimd.tensor_tensor(
                            t_sb[:, 0:128], t_sb[:, 0:128], m_sb[:, 1, :], op=OP.add
                        )

                    e_sb_full = epool.tile([128, 1152], f32, tag="e")
                    e_sb = e_sb_full[:, :wc]
                    l_sb = spool.tile([128, 1], f32, tag="l")
                    nc.scalar.activation(
                        e_sb, t_sb, AF.Exp, scale=CAP, bias=bcap, accum_out=l_sb
                    )

                    r_sb = spool.tile([128, 1], f32, tag="r")
                    nc.vector.reciprocal(r_sb, l_sb)

                    # y = E/l - 0.03/1.06  (1.06 folded into V host-side)
                    y_full = ypool.tile([128, 1152], f16, tag="y")
                    y = y_full[:, :wc]
                    nc.vector.tensor_scalar(
                        y, e_sb, r_sb, 0.03 / 1.06, op0=OP.mult, op1=OP.subtract
                    )

                    # transpose y per 128-block on PE (fp16 transpose-mode);
                    # group 4 blocks per bank-sized psum quad so DVE reads
                    # never cross a PSUM bank
                    a2_full = apool.tile([128, 1152], f16, tag="a2")
                    a2 = a2_full[:, :wc]
                    for qd in range((nW + 3) // 4):
                        nblk = min(4, nW - qd * 4)
                        quad = ptp.tile([128, 512], f16, tag="pt")
                        for wb in range(nblk):
                            w = qd * 4 + wb
                            nc.tensor.transpose(
                                quad[:, wb * 128:(wb + 1) * 128],
                                y[:, w * 128:(w + 1) * 128],
                                i_sb,
                            )
                        # clamp to [0,1] during PSUM->SBUF copy
                        nc.vector.tensor_scalar(
                            a2[:, qd * 512: qd * 512 + nblk * 128],
                            quad[:, : nblk * 128],
                            0.0,
                            1.0 / 1.06,
                            op0=OP.max,
                            op1=OP.min,
                        )

                    po = pout.tile([128, 128], f32, tag="po")
                    for w in range(nW):
                        nc.tensor.matmul(
                            po,
                            lhsT=a2[:, w * 128:(w + 1) * 128],
                            rhs=v_sb[:, j0 + w, :],
                            start=(w == 0),
                            stop=(w == nW - 1),
                        )

                    o_sb = opool.tile([128, 128], f32, tag="o")
                    nc.vector.tensor_copy(o_sb, po)
                    nc.sync.dma_start(out[i * 128:(i + 1) * 128, h, :], o_sb)

    nc.compile()
    return nc


def _host_inputs(q, k, v):
    q = np.asarray(q, dtype=np.float32)
    k = np.asarray(k, dtype=np.float32)
    v = np.asarray(v, dtype=np.float32)

    a = np.arange(128)
    mask_diag = np.where(a[None, :] <= a[:, None], 0.0, MASK_VAL).astype(np.float32)
    mask_left = np.where(a[None, :] >= a[:, None], 0.0, MASK_VAL).astype(np.float32)
    msk = np.stack([mask_diag, mask_left]).astype(np.float32)
    idn = np.eye(128, dtype=np.float16)

    in_maps = []
    for c in range(8):
        b = c // 4
        g = c % 4
        qTc = np.ascontiguousarray(
            q[b, :, 4 * g:4 * g + 4, :].transpose(1, 2, 0)
        ).astype(np.float32)
        kTh = np.ascontiguousarray(k[b, :, g, :].T).astype(np.float32)
        vhh = (np.float32(1.06) * np.ascontiguousarray(v[b, :, g, :])).astype(np.float16)
        in_maps.append({"qT": qTc, "kT": kTh, "vh": vhh, "msk": msk, "idn": idn})
    return in_maps


def _run(q, k, v, trace=False):
    from concourse.bass_utils import run_bass_kernel_spmd

    if "nc" not in _CACHED:
        _CACHED["nc"] = _build_bass()
    nc = _CACHED["nc"]

    in_maps = _host_inputs(q, k, v)
    res = run_bass_kernel_spmd(nc, in_maps, list(range(8)), trace=trace)

    out = np.zeros((B, S, HQ, D), np.float32)
    for c in range(8):
        b = c // 4
        g = c % 4
        out[b, :, 4 * g:4 * g + 4, :] = res.results[c]["out"]
    return out, res


def kernel(q, k, v):
    out, _ = _run(q, k, v, trace=False)
    return out


def kernel_traced(q, k, v):
    out, res = _run(q, k, v, trace=True)
    return out, res

